# revision 5
# baseline (speedup 1.0000x reference)
"""Trainium2 Bass kernel for BertAttention (B=16, S=1024, H=768, 12 heads).

Strategy: data-parallel over batch across 8 NeuronCores (2 batch rows per
core), no collectives.  Per core:
  - cast x / weights to bf16 in DRAM (SWDGE cast DMA), hardware DMA-transpose
    into SBUF to get contraction-dim-on-partition layouts (zero PE cost).
  - QKV projections as lhsT.T @ rhs matmuls (bf16, fp32 PSUM accum), Q/K in
    transposed [feature, token] layout, V in natural [token, feature] layout.
  - attention per head-pair: head_dim=64 so two heads run concurrently in the
    128x128 PE array via row/col tile_position packing.
  - softmax: no max-subtraction needed (scores are O(1)); exp on ScalarE with
    the 1/sqrt(64) scale folded in; additive mask applied exactly as a
    multiplicative exp(mask) folded into V rows and the denominator lhsT.
  - denominators via ones-matmul (sum over k lands broadcast across
    partitions); reciprocal + multiply folded into the PSUM->SBUF copy.
  - output projection + residual + LayerNorm (bn_stats/bn_aggr, one batched
    Sqrt per batch row to avoid ACT table thrashing with exp).

Workaround: this container's walrus accepts only ONE sync wait per
instruction; a post-pass splits multi-wait instructions into single-wait
NOPs.
"""

import numpy as np

import concourse.bass as bass
import concourse.mybir as mybir
import concourse.tile as tile
from concourse.tile import add_dep_helper

P = 128
H = 768
NH = 12
HD = 64
S = 1024
B = 16
NCORES = 8
BPC = B // NCORES  # batch rows per core = 2
IO_T = H // P      # 6 contraction tiles
KO_T = S // P      # 8 k tiles per sequence
SCALE = 1.0 / 8.0  # 1/sqrt(64)
LN_EPS = 1e-12

F32 = mybir.dt.float32
BF16 = mybir.dt.bfloat16
AF = mybir.ActivationFunctionType
OP = mybir.AluOpType


def _split_multi_waits(nc):
    """walrus here rejects >1 sync wait per instruction; hoist extras into
    single-wait NOPs on the same engine immediately before."""
    n = 0
    for blk in nc.m.functions[0].blocks:
        insts = blk.instructions
        new = []
        changed = False
        for inst in insts:
            si = inst.sync_info
            waits = list(si.on_wait) if si and si.on_wait else []
            if len(waits) > 1:
                changed = True
                for k, w in enumerate(waits[:-1]):
                    n += 1
                    new.append(
                        mybir.InstNoOp(
                            name=f"ws-{blk.name}-{inst.name}-{k}",
                            engine=inst.engine,
                            sync_info=mybir.SyncInfo(on_wait=[w], on_update=[]),
                        )
                    )
                inst.sync_info = mybir.SyncInfo(
                    on_wait=[waits[-1]], on_update=list(si.on_update)
                )
            new.append(inst)
        if changed:
            blk.instructions = new
    return n


def _bcast_ap(ap, parts=P):
    """Partition-broadcast view of a 1-D DRAM AP: [parts, len]."""
    return bass.AP(tensor=ap.tensor, offset=ap.offset, ap=[[0, parts]] + list(ap.ap))


def build_bass():
    nc = bass.Bass()

    hs = nc.declare_dram_parameter("hs", [BPC, S, H], F32, isOutput=False)
    msk = nc.declare_dram_parameter("msk", [BPC, S], F32, isOutput=False)
    qw = nc.declare_dram_parameter("qw", [H, H], F32, isOutput=False)
    kw = nc.declare_dram_parameter("kw", [H, H], F32, isOutput=False)
    vw = nc.declare_dram_parameter("vw", [H, H], F32, isOutput=False)
    ow = nc.declare_dram_parameter("ow", [H, H], F32, isOutput=False)
    qb = nc.declare_dram_parameter("qb", [H], F32, isOutput=False)
    kb = nc.declare_dram_parameter("kb", [H], F32, isOutput=False)
    vb = nc.declare_dram_parameter("vb", [H], F32, isOutput=False)
    ob = nc.declare_dram_parameter("ob", [H], F32, isOutput=False)
    gamma = nc.declare_dram_parameter("gamma", [H], F32, isOutput=False)
    beta = nc.declare_dram_parameter("beta", [H], F32, isOutput=False)
    out = nc.declare_dram_parameter("out", [BPC, S, H], F32, isOutput=True)

    from contextlib import ExitStack

    with tile.TileContext(nc) as tc:
        with ExitStack() as ctx:
            _build_tile(
                ctx, tc, nc, hs, msk, qw, kw, vw, ow, qb, kb, vb, ob, gamma, beta, out
            )

    _split_multi_waits(nc)
    return nc


def _build_tile(ctx, tc, nc, hs, msk, qw, kw, vw, ow, qb, kb, vb, ob, gamma, beta, out):
    dram = ctx.enter_context(tc.tile_pool(name="dram", bufs=1, space="DRAM"))
    consts = ctx.enter_context(tc.tile_pool(name="consts", bufs=1))
    perb = ctx.enter_context(tc.tile_pool(name="perb", bufs=1))
    xt_pool = ctx.enter_context(tc.tile_pool(name="xt", bufs=2))
    pt_pool = ctx.enter_context(tc.tile_pool(name="pt", bufs=3))
    r_pool = ctx.enter_context(tc.tile_pool(name="rcp", bufs=2))
    xres_pool = ctx.enter_context(tc.tile_pool(name="xres", bufs=2))
    s_pool = ctx.enter_context(tc.tile_pool(name="s", bufs=8))
    o_pool = ctx.enter_context(tc.tile_pool(name="o", bufs=2))
    ln_pool = ctx.enter_context(tc.tile_pool(name="ln", bufs=4))

    ps_proj = ctx.enter_context(tc.tile_pool(name="psp", bufs=2, space="PSUM"))
    ps_acc = ctx.enter_context(tc.tile_pool(name="psa", bufs=1, space="PSUM"))
    ps_big = ctx.enter_context(tc.tile_pool(name="psb", bufs=2, space="PSUM"))

    # ---- constants / weight preparation -------------------------------
    # bf16 copies of weights in DRAM, then DMA-transpose into SBUF
    wT = {}
    for name, w in (("q", qw), ("k", kw), ("v", vw), ("o", ow)):
        w_bf = dram.tile([H, H], BF16)
        nc.gpsimd.dma_start(out=w_bf, in_=w[:, :])  # cast f32 -> bf16
        t = consts.tile([P, IO_T, H], BF16, tag=f"wT_{name}")
        for io in range(IO_T):
            nc.sync.dma_start_transpose(t[:, io, :], w_bf[:, io * P : (io + 1) * P])
        wT[name] = t

    x_bf = dram.tile([BPC, S, H], BF16)
    nc.gpsimd.dma_start(out=x_bf, in_=hs[:, :, :])

    gamma_bc = consts.tile([P, H], F32, tag="gamma_bc")
    nc.gpsimd.dma_start(out=gamma_bc, in_=_bcast_ap(gamma[:]))
    beta_bc = consts.tile([P, H], F32, tag="beta_bc")
    nc.gpsimd.dma_start(out=beta_bc, in_=_bcast_ap(beta[:]))

    qb_sb = consts.tile([P, IO_T], F32, tag="qb")
    nc.sync.dma_start(out=qb_sb, in_=qb[:].rearrange("(o p) -> p o", p=P))
    kb_sb = consts.tile([P, IO_T], F32, tag="kb")
    nc.sync.dma_start(out=kb_sb, in_=kb[:].rearrange("(o p) -> p o", p=P))

    vb_row = consts.tile([1, H], BF16, tag="vb_row")
    nc.gpsimd.dma_start(out=vb_row, in_=vb[:][None, :])
    ob_row = consts.tile([1, H], BF16, tag="ob_row")
    nc.gpsimd.dma_start(out=ob_row, in_=ob[:][None, :])

    eps_sb = consts.tile([P, 1], F32, tag="eps")
    nc.vector.memset(eps_sb, LN_EPS)
    ones64 = consts.tile([P, HD], F32, tag="ones64")
    nc.vector.memset(ones64, 1.0)
    ones_row = consts.tile([1, P], BF16, tag="ones_row")
    nc.vector.memset(ones_row, 1.0)

    HP = NH // 2  # 6 head pairs
    QT_CH = 512   # q chunk (free dim of attention matmuls)
    NQ = S // QT_CH  # 2

    for b in range(BPC):
        # ---- per-b prep ------------------------------------------------
        xT = xt_pool.tile([P, IO_T, S], BF16, tag="xT")
        for io in range(IO_T):
            nc.sync.dma_start_transpose(xT[:, io, :], x_bf[b, :, io * P : (io + 1) * P])

        mask_sb = perb.tile([P, KO_T], F32, tag="mask")
        nc.sync.dma_start(out=mask_sb, in_=msk[:, :][b].rearrange("(o p) -> p o", p=P))
        em_sb = perb.tile([P, KO_T], F32, tag="em")
        nc.scalar.activation(out=em_sb, in_=mask_sb, func=AF.Exp)
        em_lhsT = perb.tile([P, KO_T, HD], BF16, tag="em_lhsT")
        for ko in range(KO_T):
            nc.vector.tensor_scalar_mul(
                out=em_lhsT[:, ko, :], in0=ones64, scalar1=em_sb[:, ko : ko + 1]
            )

        # ---- QKV projections ------------------------------------------
        QT = perb.tile([P, IO_T, S], BF16, tag="QT")
        KT = perb.tile([P, IO_T, S], BF16, tag="KT")
        for dst, wname, bias in ((QT, "q", qb_sb), (KT, "k", kb_sb)):
            for jo in range(IO_T):
                for tt in range(S // 512):
                    ps = ps_proj.tile([P, 512], F32, tag="proj")
                    for io in range(IO_T):
                        nc.tensor.matmul(
                            ps,
                            lhsT=wT[wname][:, io, jo * P : (jo + 1) * P],
                            rhs=xT[:, io, tt * 512 : (tt + 1) * 512],
                            start=(io == 0),
                            stop=(io == IO_T - 1),
                        )
                    nc.vector.tensor_scalar_add(
                        out=dst[:, jo, tt * 512 : (tt + 1) * 512],
                        in0=ps,
                        scalar1=bias[:, jo : jo + 1],
                    )

        V = perb.tile([P, KO_T, H], BF16, tag="V")
        for t8 in range(KO_T):
            for jh in range(2):
                ps = ps_proj.tile([P, 512], F32, tag="proj")
                for io in range(IO_T):
                    nc.tensor.matmul(
                        ps[:, 0:384],
                        lhsT=xT[:, io, t8 * P : (t8 + 1) * P],
                        rhs=wT["v"][:, io, jh * 384 : (jh + 1) * 384],
                        start=(io == 0),
                        stop=False,
                    )
                nc.tensor.matmul(
                    ps[:, 0:384],
                    lhsT=ones_row,
                    rhs=vb_row[:, jh * 384 : (jh + 1) * 384],
                    start=False,
                    stop=True,
                )
                # copy + exp(mask) row scaling (exact multiplicative mask)
                nc.vector.tensor_scalar_mul(
                    out=V[:, t8, jh * 384 : (jh + 1) * 384],
                    in0=ps[:, 0:384],
                    scalar1=em_sb[:, t8 : t8 + 1],
                )

        # ---- attention per head pair ----------------------------------
        ctxT = perb.tile([P, HP, S], BF16, tag="ctxT")
        for hp in range(HP):
            for qt in range(NQ):
                qsl = slice(qt * QT_CH, (qt + 1) * QT_CH)
                ptA = pt_pool.tile([P, KO_T, QT_CH], BF16, tag="ptA")
                ptB = pt_pool.tile([P, KO_T, QT_CH], BF16, tag="ptB")
                # scores^T -> exp, in chunks of 2 k-tiles (2 PSUM banks)
                for kc in range(KO_T // 2):
                    for pt_dst, lo in ((ptA, 0), (ptB, HD)):
                        sc = ps_big.tile([P, 2, 512], F32, tag="sc")
                        for k2 in range(2):
                            ko = kc * 2 + k2
                            nc.tensor.matmul(
                                sc[:, k2, :],
                                lhsT=KT[lo : lo + HD, hp, ko * P : (ko + 1) * P],
                                rhs=QT[lo : lo + HD, hp, qsl],
                                start=True,
                                stop=True,
                            )
                        nc.scalar.activation(
                            out=pt_dst[:, kc * 2 : kc * 2 + 2, :],
                            in_=sc,
                            func=AF.Exp,
                            scale=SCALE,
                        )
                # denominators + P@V, two heads col-packed per bank
                sums = ps_acc.tile([P, QT_CH], F32, tag="sums")
                ctxp = ps_acc.tile([P, QT_CH], F32, tag="ctxp")
                first_s = {}
                for ko in range(KO_T):
                    mmA = nc.tensor.matmul(
                        sums[0:HD, :],
                        lhsT=em_lhsT[:, ko, :],
                        rhs=ptA[:, ko, :],
                        start=(ko == 0),
                        stop=(ko == KO_T - 1),
                        tile_position=(0, 0),
                    )
                    mmB = nc.tensor.matmul(
                        sums[HD:P, :],
                        lhsT=em_lhsT[:, ko, :],
                        rhs=ptB[:, ko, :],
                        start=False,
                        stop=(ko == KO_T - 1),
                        tile_position=(0, HD),
                    )
                    if ko == 0:
                        add_dep_helper(mmB.ins, mmA.ins, sync=False, reason="bank clear order")
                    mmC = nc.tensor.matmul(
                        ctxp[0:HD, :],
                        lhsT=V[:, ko, hp * P : hp * P + HD],
                        rhs=ptA[:, ko, :],
                        start=(ko == 0),
                        stop=(ko == KO_T - 1),
                        tile_position=(0, 0),
                    )
                    mmD = nc.tensor.matmul(
                        ctxp[HD:P, :],
                        lhsT=V[:, ko, hp * P + HD : (hp + 1) * P],
                        rhs=ptB[:, ko, :],
                        start=False,
                        stop=(ko == KO_T - 1),
                        tile_position=(0, HD),
                    )
                    if ko == 0:
                        add_dep_helper(mmD.ins, mmC.ins, sync=False, reason="bank clear order")
                rcp = r_pool.tile([P, QT_CH], F32, tag="rcp")
                nc.vector.reciprocal(out=rcp, in_=sums)
                nc.vector.tensor_tensor(
                    out=ctxT[:, hp, qsl], in0=ctxp, in1=rcp, op=OP.mult
                )

        # ---- output projection + residual + layernorm -----------------
        mv_all = ln_pool.tile([P, KO_T, 2], F32, tag="mv")
        s_tiles = []
        for t8 in range(KO_T):
            xres = xres_pool.tile([P, H], F32, tag="xres")
            nc.sync.dma_start(out=xres, in_=hs[b, t8 * P : (t8 + 1) * P, :])
            s_t = s_pool.tile([P, H], F32, tag="s")
            for jh in range(2):
                ps = ps_proj.tile([P, 512], F32, tag="proj")
                for io in range(IO_T):
                    nc.tensor.matmul(
                        ps[:, 0:384],
                        lhsT=ctxT[:, io, t8 * P : (t8 + 1) * P],
                        rhs=wT["o"][:, io, jh * 384 : (jh + 1) * 384],
                        start=(io == 0),
                        stop=False,
                    )
                nc.tensor.matmul(
                    ps[:, 0:384],
                    lhsT=ones_row,
                    rhs=ob_row[:, jh * 384 : (jh + 1) * 384],
                    start=False,
                    stop=True,
                )
                nc.vector.tensor_tensor(
                    out=s_t[:, jh * 384 : (jh + 1) * 384],
                    in0=ps[:, 0:384],
                    in1=xres[:, jh * 384 : (jh + 1) * 384],
                    op=OP.add,
                )
            stats = ln_pool.tile([P, 3, 6], F32, tag="stats")
            for sg in range(3):
                nc.vector.bn_stats(
                    out=stats[:, sg, :], in_=s_t[:, sg * 256 : (sg + 1) * 256]
                )
            nc.vector.bn_aggr(out=mv_all[:, t8, :], in_=stats)
            s_tiles.append(s_t)

        rstd = ln_pool.tile([P, KO_T], F32, tag="rstd")
        nc.scalar.activation(
            out=rstd, in_=mv_all[:, :, 1], func=AF.Sqrt, bias=eps_sb, scale=1.0
        )
        nc.vector.reciprocal(out=rstd, in_=rstd)

        for t8 in range(KO_T):
            o_t = o_pool.tile([P, H], F32, tag="o")
            nc.vector.tensor_scalar(
                out=o_t,
                in0=s_tiles[t8],
                scalar1=mv_all[:, t8, 0:1],
                scalar2=rstd[:, t8 : t8 + 1],
                op0=OP.subtract,
                op1=OP.mult,
            )
            nc.vector.tensor_tensor(out=o_t, in0=o_t, in1=gamma_bc, op=OP.mult)
            nc.vector.tensor_tensor(out=o_t, in0=o_t, in1=beta_bc, op=OP.add)
            nc.sync.dma_start(out=out[b, t8 * P : (t8 + 1) * P, :], in_=o_t)


_nc_cache = None


def _get_nc():
    global _nc_cache
    if _nc_cache is None:
        _nc_cache = build_bass()
    return _nc_cache


def kernel(**inputs):
    from concourse.bass_utils import run_bass_kernel_spmd

    hs = np.asarray(inputs["hidden_states"], np.float32)
    mask = np.asarray(inputs["attention_mask"], np.float32).reshape(B, S)
    names = {
        "qw": inputs["qw"], "kw": inputs["kw"], "vw": inputs["vw"], "ow": inputs["ow"],
        "qb": inputs["qb"], "kb": inputs["kb"], "vb": inputs["vb"], "ob": inputs["ob"],
        "gamma": inputs["gamma"], "beta": inputs["beta"],
    }
    shared = {k: np.ascontiguousarray(np.asarray(v, np.float32)) for k, v in names.items()}
    in_maps = []
    for c in range(NCORES):
        m = dict(shared)
        m["hs"] = np.ascontiguousarray(hs[c * BPC : (c + 1) * BPC])
        m["msk"] = np.ascontiguousarray(mask[c * BPC : (c + 1) * BPC])
        in_maps.append(m)

    nc = _get_nc()
    res = run_bass_kernel_spmd(nc, in_maps, core_ids=list(range(NCORES)))
    return np.concatenate([res.results[c]["out"] for c in range(NCORES)], axis=0)
